# revision 34
# baseline (speedup 1.0000x reference)
"""GQA attention (RoPE + softmax + o_proj) on 8 Trainium2 NeuronCores.

Problem shapes (hardcoded): hidden_states [4, 2048, 2048], 16 q heads,
4 kv heads, head_dim 128, rope cos/sin tables given as inputs.

Sharding: core c -> (batch b = c // 2, q-head half = c % 2).  Each core
computes 8 q heads + their 2 kv heads for one batch and produces a
partial o_proj output [2048, 2048]; the host sums the two halves per
batch (tensor parallel, no device collectives).

All matmuls run in fp16 (1 cycle/row on PE) with fp32 PSUM accumulation.
Phase A (projections + RoPE) is a software-pipelined chain sequence;
RoPE is applied by DVE (partition shuffle + mul/add against host-
permuted cos/sin tables).

Phase B emits one "iteration" per (head, s-block): a fine-grained
round-robin PE stream interleaving
  - the scores matmul pairs for (h, si) (each pair immediately followed
    by a ScalarE exp that writes P^T fp16 to SBUF),
  - the P@V accumulation chain for the previous iteration's block, and
  - two o_proj chains (spread across the following s-block's iterations
    instead of running as a burst),
so the PE never idles waiting on ScalarE and stays at full clock (the
TensorE down-clocks after pipeline gaps and takes ~3us to re-ramp).
Softmax denominators come from a DVE pairwise-add tree plus a GpSimd
partition_all_reduce (replicated output), freeing the PE of the
all-ones matmul and a PSUM bank.  o_proj copyback runs on DVE (fp16)
and the kernel output is fp16, summed in fp32 on the host.
"""

import sys

import numpy as np

B, S, HID = 4, 2048, 2048
NH, NKV, HD = 16, 4, 128
NH_L = 8        # q heads per core
NKV_L = 2       # kv heads per core
GROUP = NH // NKV
P = 128
ST = 512        # s-block (matmul free dim)
NSB = S // ST   # 4 s-blocks
KT = HID // P   # 16 contraction tiles over hidden
TT = S // P     # 16 key/t tiles
SCALE = 1.0 / float(np.sqrt(HD))

_CACHE = {}


def _build():
    if "/opt/trn_rl_repo" not in sys.path:
        sys.path.insert(0, "/opt/trn_rl_repo")
    import concourse.mybir as mybir
    from concourse import bacc
    from concourse import bass_isa
    from concourse.tile import TileContext
    from concourse.tile_rust import add_dep_helper

    dt = mybir.dt
    f16, f32 = dt.float16, dt.float32

    nc = bacc.Bacc("TRN2", target_bir_lowering=False, debug=False, num_devices=8)
    # host-pretiled layouts (see kernel() below)
    hsT = nc.dram_tensor("hsT", [P, NSB, KT, ST], f16, kind="ExternalInput").ap()
    wq = nc.dram_tensor("wq", [P, NH_L, KT, HD], f16, kind="ExternalInput").ap()
    wk = nc.dram_tensor("wk", [P, NKV_L, KT, HD], f16, kind="ExternalInput").ap()
    wv = nc.dram_tensor("wv", [P, KT, NKV_L * HD], f16, kind="ExternalInput").ap()
    wo = nc.dram_tensor("wo", [P, NH_L, HID], f16, kind="ExternalInput").ap()
    cosT = nc.dram_tensor("cosT", [HD, S], f16, kind="ExternalInput").ap()
    sinT = nc.dram_tensor("sinT", [HD, S], f16, kind="ExternalInput").ap()
    out = nc.dram_tensor("out", [S, HID], f16, kind="ExternalOutput").ap()

    EXP = mybir.ActivationFunctionType.Exp

    with TileContext(nc) as tc:
        with (
            tc.tile_pool(name="consts", bufs=1) as consts,
            tc.tile_pool(name="qkv", bufs=1) as qkvp,
            tc.tile_pool(name="trig", bufs=1) as trig,
            tc.tile_pool(name="wqhi", bufs=1) as wqhip,
            tc.tile_pool(name="hs3", bufs=1) as hs3p,
        ):
            ones = consts.tile([P, P], f16, tag="ones")
            nc.vector.memset(ones, 1.0)
            # rotate_half as an intra-quadrant partition shuffle (the head
            # dim is host-permuted so +-64 pairs sit 16 apart per quadrant;
            # the sign lives in the pre-negated sin table)
            SHUF = list(range(16, 32)) + list(range(0, 16))

            q_sb = qkvp.tile([P, NH_L, S], f16, tag="q")
            k_sb = qkvp.tile([P, NKV_L, S], f16, tag="k")
            v_sb = qkvp.tile([P, TT, NKV_L * HD], f16, tag="v")

            # ---------------- Phase A: projections + RoPE ----------------
            with (
                tc.tile_pool(name="wqkv", bufs=1) as wp,
                tc.tile_pool(name="hs", bufs=2) as hsp,
                tc.tile_pool(name="ropes", bufs=4) as smalls,
                tc.tile_pool(name="psA", bufs=6, space="PSUM") as psA,
            ):
                # hs block 0 first (its consumers are the head of the program)
                hs_blks = {}
                # first hs block + wv arrive in interleaved chunks so the
                # very first projection group starts after ~1MB, not ~3MB
                hs_first = hsp.tile([P, KT, ST], f16, tag="hs")
                wv_sb = wp.tile([P, KT, NKV_L * HD], f16, tag="wv")
                wk_sb = wp.tile([P, NKV_L, KT, HD], f16, tag="wk")
                # hs0 as two large DMAs on parallel queues: 8KB-per-partition
                # descriptors move ~3x faster than small interleaved chunks.
                # wk (small, first-needed weight) rides the sync queue first,
                # then wv; k chains run before v chains in block 0.
                nc.scalar.dma_start(out=hs_first[:, 0:6, :], in_=hsT[:, 0, 0:6, :])
                hs0_dma = nc.gpsimd.dma_start(
                    out=hs_first[:, 6:11, :], in_=hsT[:, 0, 6:11, :]
                )
                nc.sync.dma_start(out=hs_first[:, 11:16, :], in_=hsT[:, 0, 11:16, :])
                nc.scalar.dma_start(out=wk_sb[:, 0, :, :], in_=wk[:, 0, :, :])
                nc.scalar.dma_start(out=wk_sb[:, 1, :, :], in_=wk[:, 1, :, :])
                nc.sync.dma_start(out=wv_sb[:, 0:8, :], in_=wv[:, 0:8, :])
                nc.sync.dma_start(out=wv_sb[:, 8:16, :], in_=wv[:, 8:16, :])
                hs_blks[0] = hs_first
                hs_dmas = [hs0_dma]

                # cos/sin on the scalar queue (free after its hs0 half) so
                # they don't delay hs1 on gpsimd
                cos_sb = trig.tile([HD, S], f16, tag="cos")
                nc.scalar.dma_start(out=cos_sb, in_=cosT)
                sin_sb = trig.tile([HD, S], f16, tag="sin")
                nc.scalar.dma_start(out=sin_sb, in_=sinT)
                # q-weight split: heads 0-1 die with phase A; heads 2-7 stay
                # resident for the deferred block-3 q chains in phase B
                wq_sb = wp.tile([P, 2, KT, HD], f16, tag="wq")
                wqhi_sb = wqhip.tile([P, NH_L - 2, KT, HD], f16, tag="wqhi")

                def wq_slice(h):
                    return wq_sb[:, h] if h < 2 else wqhi_sb[:, h - 2]

                for h in range(NH_L):  # per-head DMAs so head 0 starts early
                    wqd = nc.sync.dma_start(out=wq_slice(h), in_=wq[:, h, :, :])
                    # keep HBM bandwidth free for the first-needed tensors
                    add_dep_helper(
                        wqd.ins, hs0_dma.ins, sync=True, reason="defer wq behind hs0"
                    )

                # software pipeline: the rot-shuffle + rope combine for one
                # projection is emitted while the NEXT projection's matmul
                # group runs, so PE never waits on the PSUM copyback.
                pending = []

                def rope_flush():
                    qc, s0, dst, dsti = pending.pop(0)
                    rc = smalls.tile([P, ST], f16, tag="rc")
                    nc.vector.stream_shuffle(rc, qc, SHUF)
                    t1 = smalls.tile([P, ST], f16, tag="t1")
                    nc.vector.tensor_mul(t1, qc, cos_sb[:, s0 : s0 + ST])
                    t2 = smalls.tile([P, ST], f16, tag="t2")
                    nc.vector.tensor_mul(t2, rc, sin_sb[:, s0 : s0 + ST])
                    nc.vector.tensor_add(dst[:, dsti, s0 : s0 + ST], t1, t2)

                for si in range(NSB):
                    s0 = si * ST
                    if si in hs_blks:
                        hs_blk = hs_blks[si]
                    else:
                        # block 3 stays resident for the deferred q chains
                        pool = hs3p if si == NSB - 1 else hsp
                        hs_blk = pool.tile([P, KT, ST], f16, tag="hs")
                        hs_blks[si] = hs_blk
                        # gpsimd queue is otherwise idle, so chaining these
                        # issues behind the previous block stalls nothing
                        hd = nc.gpsimd.dma_start(out=hs_blk, in_=hsT[:, si, :, :])
                        add_dep_helper(
                            hd.ins,
                            hs_dmas[-1].ins,
                            sync=True,
                            reason="stagger hs blocks",
                        )
                        hs_dmas.append(hd)

                    def proj(w_slice, dst, dsti):
                        pm = psA.tile([P, ST], f32, tag="ps")
                        for kt in range(KT):
                            nc.tensor.matmul(
                                pm,
                                lhsT=w_slice[:, kt, :],
                                rhs=hs_blk[:, kt, :],
                                start=(kt == 0),
                                stop=(kt == KT - 1),
                            )
                        qc = smalls.tile([P, ST], f16, tag="qc")
                        nc.vector.tensor_copy(qc, pm)
                        pending.append((qc, s0, dst, dsti))

                    def v_chains():
                        for sj in range(ST // P):
                            tt = s0 // P + sj
                            pv = psA.tile([P, NKV_L * HD], f32, tag="ps")
                            for kt in range(KT):
                                nc.tensor.matmul(
                                    pv,
                                    lhsT=hs_blk[:, kt, sj * P : (sj + 1) * P],
                                    rhs=wv_sb[:, kt, :],
                                    start=(kt == 0),
                                    stop=(kt == KT - 1),
                                )
                            nc.scalar.copy(v_sb[:, tt, :], pv)

                    # block 0: k first (wk is the smallest weight, so the
                    # first chains need the least data in flight)
                    if si != 0:
                        v_chains()
                    for j in range(NKV_L):
                        proj(wk_sb[:, j], k_sb, j)
                        if len(pending) > 1:
                            rope_flush()
                    if si == 0:
                        v_chains()
                    # block 3: heads 2-7 are deferred into phase B's light
                    # first s-block iterations (ScalarE-paced otherwise)
                    nq = 2 if si == NSB - 1 else NH_L
                    for h in range(nq):
                        proj(wq_slice(h), q_sb, h)
                        if len(pending) > 1:
                            rope_flush()
                while pending:
                    rope_flush()

            # ---------------- Phase B: attention + interleaved o_proj ------
            with (
                tc.tile_pool(name="wo", bufs=1) as wop,
                tc.tile_pool(name="attn", bufs=1) as ap_,
                tc.tile_pool(name="pblk", bufs=2) as pp,
                tc.tile_pool(name="accs", bufs=2) as accp,
                tc.tile_pool(name="tmps", bufs=2) as tmpp,
                tc.tile_pool(name="rcps", bufs=2) as rcpp,
                tc.tile_pool(name="flush", bufs=1) as flushp,
                tc.tile_pool(name="outp", bufs=2) as op_,
                tc.tile_pool(name="psc", bufs=2, space="PSUM") as pscp,
                tc.tile_pool(name="pat", bufs=1, space="PSUM") as patp,
                tc.tile_pool(name="pcs", bufs=1, space="PSUM") as pcsp,
                tc.tile_pool(name="po", bufs=2, space="PSUM") as pop,
            ):
                wo_sb = wop.tile([P, NH_L, HID], f16, tag="wo")
                wod = nc.sync.dma_start(out=wo_sb, in_=wo)
                add_dep_helper(
                    wod.ins, hs0_dma.ins, sync=True, reason="defer wo behind hs0"
                )
                # attnT ring over s-blocks: slot si%2 (o_proj of block si
                # finishes before block si+2 starts normalizing)
                attnT = ap_.tile([P, NH_L, 2, ST], f16, tag="attnT")

                # o_proj chain bookkeeping: one chain per (si, sj, ni);
                # ob rows accumulate 4 ni copies then DMA out per sj.
                ob_tiles = {}

                def o_chain(si, sj, ni):
                    po = pop.tile([P, ST], f32, tag="po")
                    for ft in range(NH_L):
                        nc.tensor.matmul(
                            po,
                            lhsT=attnT[:, ft, si % 2, sj * P : (sj + 1) * P],
                            rhs=wo_sb[:, ft, ni * ST : (ni + 1) * ST],
                            start=(ft == 0),
                            stop=(ft == NH_L - 1),
                        )
                    return po

                def o_finish(si, sj, ni, po):
                    st = si * (ST // P) + sj
                    if ni == 0:
                        ob_tiles[(si, sj)] = op_.tile(
                            [P, HID], f16, tag="ob", name="ob"
                        )
                    ob = ob_tiles[(si, sj)]
                    nc.vector.tensor_copy(ob[:, ni * ST : (ni + 1) * ST], po)
                    if si == NSB - 1 and sj >= 2:
                        # stream the last rows per-ni so the final DMA isn't
                        # serialized behind all four copybacks
                        nc.sync.dma_start(
                            out=out[st * P : (st + 1) * P, ni * ST : (ni + 1) * ST],
                            in_=ob[:, ni * ST : (ni + 1) * ST],
                        )
                        if ni == NSB - 1:
                            del ob_tiles[(si, sj)]
                    elif ni == NSB - 1:
                        nc.sync.dma_start(
                            out=out[st * P : (st + 1) * P, :], in_=ob
                        )
                        del ob_tiles[(si, sj)]

                # softmax denominator: DVE accumulates pairwise sums of the
                # exp tiles as they complete (latency after the last exp is
                # one pair-add + one acc-add), then a single all-ones matmul
                # reduces across partitions with the result replicated.
                def acc_pair(acc, pblk, k):
                    """add exp tile pair k into the running accumulator."""
                    if k == 0:
                        nc.vector.tensor_add(
                            acc, pblk[:, 0, :], pblk[:, 1, :]
                        )
                    else:
                        tmp = tmpp.tile([P, ST], f16, tag="tmp")
                        nc.vector.tensor_add(
                            tmp, pblk[:, 2 * k, :], pblk[:, 2 * k + 1, :]
                        )
                        nc.vector.tensor_add(acc, acc, tmp)

                def denom_reduce(acc):
                    """partition-sum of acc via ones-matmul; returns rcp."""
                    pcs = pcsp.tile([P, ST], f32, tag="pcs")
                    nc.tensor.matmul(pcs, lhsT=ones, rhs=acc, start=True, stop=True)
                    rcp = rcpp.tile([P, ST], f32, tag="rcp")
                    nc.vector.reciprocal_approx_fast(out=rcp, in_=pcs)
                    return rcp

                def normalize(h, si, pat, rcp):
                    nc.vector.tensor_mul(attnT[:, h, si % 2, :], pat, rcp)

                # iteration t: scores for blk[t], P@V+post for blk[t-1],
                # o_proj chains per OSCHED.
                blks = [(h, si) for si in range(NSB) for h in range(NH_L)]
                chains = {
                    si: [(si, sj, ni) for sj in range(ST // P) for ni in range(NSB)]
                    for si in range(NSB)
                }
                OSCHED = {t: [] for t in range(len(blks) + 1)}
                for si in range(NSB):
                    # chains 2c,2c+1 of o_proj(si) at iteration (si+1)*8+1+c,
                    # chains 14,15 at iteration (si+2)*8; overflow -> tail.
                    for c in range(7):
                        t = (si + 1) * 8 + 1 + c
                        OSCHED[min(t, len(blks))] += chains[si][2 * c : 2 * c + 2]
                    t = (si + 2) * 8
                    OSCHED[min(t, len(blks))] += chains[si][14:16]

                prev = None  # (h, si, pblk, acc)
                for t, (h, si) in enumerate(blks):
                    j = h // GROUP
                    s0 = si * ST
                    pblk = pp.tile([P, TT, ST], f16, tag="pblk")
                    acc = accp.tile([P, ST], f16, tag="acc")
                    ochains = OSCHED[t]
                    pat = None
                    rcp = None
                    po_a = po_b = None
                    for k in range(8):
                        # scores pair k -> exp
                        psc = pscp.tile([P, 2, ST], f32, tag="psc")
                        for u in range(2):
                            tt = 2 * k + u
                            nc.tensor.matmul(
                                psc[:, u, :],
                                lhsT=k_sb[:, j, tt * P : (tt + 1) * P],
                                rhs=q_sb[:, h, s0 : s0 + ST],
                                start=True,
                                stop=True,
                            )
                        nc.scalar.activation(
                            out=pblk[:, 2 * k : 2 * k + 2, :],
                            in_=psc,
                            func=EXP,
                            scale=SCALE,
                        )
                        # 2 P@V chain matmuls for the previous block
                        if prev is not None:
                            ph_, psi_, pblk_prev, acc_prev = prev
                            jp = ph_ // GROUP
                            if k == 0:
                                pat = patp.tile([P, ST], f32, tag="pat")
                                acc_pair(acc_prev, pblk_prev, 7)
                            for u in range(2):
                                tt = 2 * k + u
                                nc.tensor.matmul(
                                    pat,
                                    lhsT=v_sb[:, tt, jp * HD : (jp + 1) * HD],
                                    rhs=pblk_prev[:, tt, :],
                                    start=(tt == 0),
                                    stop=(tt == TT - 1),
                                )
                            if k == 2:
                                rcp = denom_reduce(acc_prev)
                            if k == 7:
                                normalize(ph_, psi_, pat, rcp)
                        # running denominator partial sums for this block
                        if k >= 1:
                            acc_pair(acc, pblk, k - 1)
                        # 2 o_proj chain matmuls (chain A at k<4, B at k>=4)
                        if ochains:
                            if k < 4:
                                if k == 0:
                                    po_a = pop.tile([P, ST], f32, tag="po")
                                ca = ochains[0]
                                for u in range(2):
                                    ft = 2 * k + u
                                    nc.tensor.matmul(
                                        po_a,
                                        lhsT=attnT[
                                            :, ft, ca[0] % 2,
                                            ca[1] * P : (ca[1] + 1) * P,
                                        ],
                                        rhs=wo_sb[
                                            :, ft, ca[2] * ST : (ca[2] + 1) * ST
                                        ],
                                        start=(ft == 0),
                                        stop=(ft == NH_L - 1),
                                    )
                                if k == 3:
                                    o_finish(*ochains[0], po_a)
                            elif len(ochains) > 1:
                                if k == 4:
                                    po_b = pop.tile([P, ST], f32, tag="po")
                                cb = ochains[1]
                                for u in range(2):
                                    ft = 2 * (k - 4) + u
                                    nc.tensor.matmul(
                                        po_b,
                                        lhsT=attnT[
                                            :, ft, cb[0] % 2,
                                            cb[1] * P : (cb[1] + 1) * P,
                                        ],
                                        rhs=wo_sb[
                                            :, ft, cb[2] * ST : (cb[2] + 1) * ST
                                        ],
                                        start=(ft == 0),
                                        stop=(ft == NH_L - 1),
                                    )
                                if k == 7:
                                    o_finish(*ochains[1], po_b)
                        # deferred block-3 q-projection chain (fills the
                        # otherwise ScalarE-paced first s-block iterations)
                        if 2 <= t <= 7:
                            hq = t
                            if k == 0:
                                pq = pop.tile([P, ST], f32, tag="po", name="pq")
                            for u in range(2):
                                kt = 2 * k + u
                                nc.tensor.matmul(
                                    pq,
                                    lhsT=wqhi_sb[:, hq - 2, kt, :],
                                    rhs=hs_blks[NSB - 1][:, kt, :],
                                    start=(kt == 0),
                                    stop=(kt == KT - 1),
                                )
                            if k == 7:
                                qc = flushp.tile([P, ST], f16, tag="fq")
                                nc.vector.tensor_copy(qc, pq)
                                rc = flushp.tile([P, ST], f16, tag="fr")
                                nc.vector.stream_shuffle(rc, qc, SHUF)
                                t1 = flushp.tile([P, ST], f16, tag="f1")
                                nc.vector.tensor_mul(
                                    t1, qc, cos_sb[:, (NSB - 1) * ST :]
                                )
                                t2 = flushp.tile([P, ST], f16, tag="f2")
                                nc.vector.tensor_mul(
                                    t2, rc, sin_sb[:, (NSB - 1) * ST :]
                                )
                                nc.vector.tensor_add(
                                    q_sb[:, hq, (NSB - 1) * ST :], t1, t2
                                )
                    prev = (h, si, pblk, acc)

                # ---- tail: P@V/post of last block + remaining o_proj ----
                ph_, psi_, pblk_prev, acc_prev = prev
                jp = ph_ // GROUP
                pat = patp.tile([P, ST], f32, tag="pat")
                acc_pair(acc_prev, pblk_prev, 7)
                rcp = denom_reduce(acc_prev)
                for tt in range(TT):
                    nc.tensor.matmul(
                        pat,
                        lhsT=v_sb[:, tt, jp * HD : (jp + 1) * HD],
                        rhs=pblk_prev[:, tt, :],
                        start=(tt == 0),
                        stop=(tt == TT - 1),
                    )
                normalize(ph_, psi_, pat, rcp)
                for ch in OSCHED[len(blks)]:
                    po = o_chain(*ch)
                    o_finish(*ch, po)

    nc.compile()
    return nc


def _get_nc():
    if "nc" not in _CACHE:
        _CACHE["nc"] = _build()
    return _CACHE["nc"]


def kernel(hidden_states, cos, sin, Wq, Wk, Wv, Wo):
    if "/opt/trn_rl_repo" not in sys.path:
        sys.path.insert(0, "/opt/trn_rl_repo")
    from concourse.bass_utils import run_bass_kernel_spmd

    hidden_states = np.asarray(hidden_states, dtype=np.float32)
    cos = np.asarray(cos, dtype=np.float32)
    sin = np.asarray(sin, dtype=np.float32)
    Wq = np.asarray(Wq, dtype=np.float32)
    Wk = np.asarray(Wk, dtype=np.float32)
    Wv = np.asarray(Wv, dtype=np.float32)
    Wo = np.asarray(Wo, dtype=np.float32)

    nc = _get_nc()
    dperm = np.concatenate(
        [np.r_[16 * q : 16 * q + 16, 64 + 16 * q : 64 + 16 * q + 16] for q in range(4)]
    )
    dsign = np.where(np.arange(HD) % 32 < 16, -1.0, 1.0).astype(np.float32)

    # pretiled host layouts: partition index first, contiguous per DMA slice
    def tile_khid(w):  # [HID, F] -> [P, KT, F]
        return np.ascontiguousarray(
            w.reshape(KT, P, w.shape[1]).transpose(1, 0, 2)
        ).astype(np.float16)

    in_maps = []
    hsT_b = [
        np.ascontiguousarray(
            hidden_states[b].T.reshape(KT, P, NSB, ST).transpose(1, 2, 0, 3)
        ).astype(np.float16)
        for b in range(B)
    ]
    cosT_b = [np.ascontiguousarray(cos[b].T[dperm]).astype(np.float16) for b in range(B)]
    sinT_b = [
        np.ascontiguousarray(sin[b].T[dperm] * dsign[:, None]).astype(np.float16)
        for b in range(B)
    ]
    for c in range(2 * B):
        b, half = c // 2, c % 2
        fq = slice(half * NH_L * HD, (half + 1) * NH_L * HD)
        fkv = slice(half * NKV_L * HD, (half + 1) * NKV_L * HD)
        wq_t = tile_khid(Wq[:, fq]).reshape(P, KT, NH_L, HD).transpose(0, 2, 1, 3)
        wq_t = wq_t[:, :, :, dperm]
        wk_t = tile_khid(Wk[:, fkv]).reshape(P, KT, NKV_L, HD).transpose(0, 2, 1, 3)
        wk_t = wk_t[:, :, :, dperm]
        wo_t = np.ascontiguousarray(
            Wo[fq, :].reshape(NH_L, P, HID).transpose(1, 0, 2)
        ).astype(np.float16)
        in_maps.append(
            {
                "hsT": hsT_b[b],
                "wq": np.ascontiguousarray(wq_t),
                "wk": np.ascontiguousarray(wk_t),
                "wv": tile_khid(Wv[:, fkv]),
                "wo": wo_t,
                "cosT": cosT_b[b],
                "sinT": sinT_b[b],
            }
        )

    res = run_bass_kernel_spmd(nc, in_maps, list(range(2 * B)))
    _CACHE["last_results"] = res

    out = np.empty((B, S, HID), dtype=np.float32)
    for b in range(B):
        out[b] = res.results[2 * b]["out"].astype(np.float32) + res.results[
            2 * b + 1
        ]["out"].astype(np.float32)
    return out


# revision 35
# speedup vs baseline: 1.0033x; 1.0033x over previous
"""GQA attention (RoPE + softmax + o_proj) on 8 Trainium2 NeuronCores.

Problem shapes (hardcoded): hidden_states [4, 2048, 2048], 16 q heads,
4 kv heads, head_dim 128, rope cos/sin tables given as inputs.

Sharding: core c -> (batch b = c // 2, q-head half = c % 2).  Each core
computes 8 q heads + their 2 kv heads for one batch and produces a
partial o_proj output [2048, 2048]; the host sums the two halves per
batch (tensor parallel, no device collectives).

All matmuls run in fp16 (1 cycle/row on PE) with fp32 PSUM accumulation.
Phase A (projections + RoPE) is a software-pipelined chain sequence;
RoPE is applied by DVE (partition shuffle + mul/add against host-
permuted cos/sin tables).

Phase B emits one "iteration" per (head, s-block): a fine-grained
round-robin PE stream interleaving
  - the scores matmul pairs for (h, si) (each pair immediately followed
    by a ScalarE exp that writes P^T fp16 to SBUF),
  - the P@V accumulation chain for the previous iteration's block, and
  - two o_proj chains (spread across the following s-block's iterations
    instead of running as a burst),
so the PE never idles waiting on ScalarE and stays at full clock (the
TensorE down-clocks after pipeline gaps and takes ~3us to re-ramp).
Softmax denominators come from a DVE pairwise-add tree plus a GpSimd
partition_all_reduce (replicated output), freeing the PE of the
all-ones matmul and a PSUM bank.  o_proj copyback runs on DVE (fp16)
and the kernel output is fp16, summed in fp32 on the host.
"""

import sys

import numpy as np

B, S, HID = 4, 2048, 2048
NH, NKV, HD = 16, 4, 128
NH_L = 8        # q heads per core
NKV_L = 2       # kv heads per core
GROUP = NH // NKV
P = 128
ST = 512        # s-block (matmul free dim)
NSB = S // ST   # 4 s-blocks
KT = HID // P   # 16 contraction tiles over hidden
TT = S // P     # 16 key/t tiles
SCALE = 1.0 / float(np.sqrt(HD))

_CACHE = {}


def _build():
    if "/opt/trn_rl_repo" not in sys.path:
        sys.path.insert(0, "/opt/trn_rl_repo")
    import concourse.mybir as mybir
    from concourse import bacc
    from concourse import bass_isa
    from concourse.tile import TileContext
    from concourse.tile_rust import add_dep_helper

    dt = mybir.dt
    f16, f32 = dt.float16, dt.float32

    nc = bacc.Bacc("TRN2", target_bir_lowering=False, debug=False, num_devices=8)
    # host-pretiled layouts (see kernel() below)
    hsT = nc.dram_tensor("hsT", [P, NSB, KT, ST], f16, kind="ExternalInput").ap()
    wq = nc.dram_tensor("wq", [P, NH_L, KT, HD], f16, kind="ExternalInput").ap()
    wk = nc.dram_tensor("wk", [P, NKV_L, KT, HD], f16, kind="ExternalInput").ap()
    wv = nc.dram_tensor("wv", [P, KT, NKV_L * HD], f16, kind="ExternalInput").ap()
    wo = nc.dram_tensor("wo", [P, NH_L, HID], f16, kind="ExternalInput").ap()
    cosT = nc.dram_tensor("cosT", [HD, S], f16, kind="ExternalInput").ap()
    sinT = nc.dram_tensor("sinT", [HD, S], f16, kind="ExternalInput").ap()
    out = nc.dram_tensor("out", [S, HID], f16, kind="ExternalOutput").ap()

    EXP = mybir.ActivationFunctionType.Exp

    with TileContext(nc) as tc:
        with (
            tc.tile_pool(name="consts", bufs=1) as consts,
            tc.tile_pool(name="qkv", bufs=1) as qkvp,
            tc.tile_pool(name="trig", bufs=1) as trig,
            tc.tile_pool(name="wqhi", bufs=1) as wqhip,
            tc.tile_pool(name="hs3", bufs=1) as hs3p,
        ):
            ones = consts.tile([P, P], f16, tag="ones")
            nc.vector.memset(ones, 1.0)
            # rotate_half as an intra-quadrant partition shuffle (the head
            # dim is host-permuted so +-64 pairs sit 16 apart per quadrant;
            # the sign lives in the pre-negated sin table)
            SHUF = list(range(16, 32)) + list(range(0, 16))

            q_sb = qkvp.tile([P, NH_L, S], f16, tag="q")
            k_sb = qkvp.tile([P, NKV_L, S], f16, tag="k")
            v_sb = qkvp.tile([P, TT, NKV_L * HD], f16, tag="v")

            # ---------------- Phase A: projections + RoPE ----------------
            with (
                tc.tile_pool(name="wqkv", bufs=1) as wp,
                tc.tile_pool(name="hs", bufs=2) as hsp,
                tc.tile_pool(name="ropes", bufs=4) as smalls,
                tc.tile_pool(name="psA", bufs=6, space="PSUM") as psA,
            ):
                # hs block 0 first (its consumers are the head of the program)
                hs_blks = {}
                # first hs block + wv arrive in interleaved chunks so the
                # very first projection group starts after ~1MB, not ~3MB
                hs_first = hsp.tile([P, KT, ST], f16, tag="hs")
                wv_sb = wp.tile([P, KT, NKV_L * HD], f16, tag="wv")
                wk_sb = wp.tile([P, NKV_L, KT, HD], f16, tag="wk")
                # hs0 as two large DMAs on parallel queues: 8KB-per-partition
                # descriptors move ~3x faster than small interleaved chunks.
                # wk (small, first-needed weight) rides the sync queue first,
                # then wv; k chains run before v chains in block 0.
                nc.sync.dma_start(out=wk_sb[:, 0, :, :], in_=wk[:, 0, :, :])
                nc.scalar.dma_start(out=hs_first[:, 0:6, :], in_=hsT[:, 0, 0:6, :])
                hs0_dma = nc.gpsimd.dma_start(
                    out=hs_first[:, 6:11, :], in_=hsT[:, 0, 6:11, :]
                )
                nc.sync.dma_start(out=wk_sb[:, 1, :, :], in_=wk[:, 1, :, :])
                nc.sync.dma_start(out=hs_first[:, 11:16, :], in_=hsT[:, 0, 11:16, :])
                nc.sync.dma_start(out=wv_sb[:, 0:8, :], in_=wv[:, 0:8, :])
                nc.sync.dma_start(out=wv_sb[:, 8:16, :], in_=wv[:, 8:16, :])
                hs_blks[0] = hs_first
                hs_dmas = [hs0_dma]

                # cos/sin on the scalar queue (free after its hs0 half) so
                # they don't delay hs1 on gpsimd
                cos_sb = trig.tile([HD, S], f16, tag="cos")
                nc.scalar.dma_start(out=cos_sb, in_=cosT)
                sin_sb = trig.tile([HD, S], f16, tag="sin")
                nc.scalar.dma_start(out=sin_sb, in_=sinT)
                # q-weight split: heads 0-1 die with phase A; heads 2-7 stay
                # resident for the deferred block-3 q chains in phase B
                wq_sb = wp.tile([P, 2, KT, HD], f16, tag="wq")
                wqhi_sb = wqhip.tile([P, NH_L - 2, KT, HD], f16, tag="wqhi")

                def wq_slice(h):
                    return wq_sb[:, h] if h < 2 else wqhi_sb[:, h - 2]

                for h in range(NH_L):  # per-head DMAs so head 0 starts early
                    wqd = nc.sync.dma_start(out=wq_slice(h), in_=wq[:, h, :, :])
                    # keep HBM bandwidth free for the first-needed tensors
                    add_dep_helper(
                        wqd.ins, hs0_dma.ins, sync=True, reason="defer wq behind hs0"
                    )

                # software pipeline: the rot-shuffle + rope combine for one
                # projection is emitted while the NEXT projection's matmul
                # group runs, so PE never waits on the PSUM copyback.
                pending = []

                def rope_flush():
                    qc, s0, dst, dsti = pending.pop(0)
                    rc = smalls.tile([P, ST], f16, tag="rc")
                    nc.vector.stream_shuffle(rc, qc, SHUF)
                    t1 = smalls.tile([P, ST], f16, tag="t1")
                    nc.vector.tensor_mul(t1, qc, cos_sb[:, s0 : s0 + ST])
                    t2 = smalls.tile([P, ST], f16, tag="t2")
                    nc.vector.tensor_mul(t2, rc, sin_sb[:, s0 : s0 + ST])
                    nc.vector.tensor_add(dst[:, dsti, s0 : s0 + ST], t1, t2)

                for si in range(NSB):
                    s0 = si * ST
                    if si in hs_blks:
                        hs_blk = hs_blks[si]
                    else:
                        # block 3 stays resident for the deferred q chains
                        pool = hs3p if si == NSB - 1 else hsp
                        hs_blk = pool.tile([P, KT, ST], f16, tag="hs")
                        hs_blks[si] = hs_blk
                        # gpsimd queue is otherwise idle, so chaining these
                        # issues behind the previous block stalls nothing
                        hd = nc.gpsimd.dma_start(out=hs_blk, in_=hsT[:, si, :, :])
                        add_dep_helper(
                            hd.ins,
                            hs_dmas[-1].ins,
                            sync=True,
                            reason="stagger hs blocks",
                        )
                        hs_dmas.append(hd)

                    def proj(w_slice, dst, dsti):
                        pm = psA.tile([P, ST], f32, tag="ps")
                        for kt in range(KT):
                            nc.tensor.matmul(
                                pm,
                                lhsT=w_slice[:, kt, :],
                                rhs=hs_blk[:, kt, :],
                                start=(kt == 0),
                                stop=(kt == KT - 1),
                            )
                        qc = smalls.tile([P, ST], f16, tag="qc")
                        nc.vector.tensor_copy(qc, pm)
                        pending.append((qc, s0, dst, dsti))

                    def v_chains():
                        for sj in range(ST // P):
                            tt = s0 // P + sj
                            pv = psA.tile([P, NKV_L * HD], f32, tag="ps")
                            for kt in range(KT):
                                nc.tensor.matmul(
                                    pv,
                                    lhsT=hs_blk[:, kt, sj * P : (sj + 1) * P],
                                    rhs=wv_sb[:, kt, :],
                                    start=(kt == 0),
                                    stop=(kt == KT - 1),
                                )
                            nc.scalar.copy(v_sb[:, tt, :], pv)

                    # block 0: k first (wk is the smallest weight, so the
                    # first chains need the least data in flight)
                    if si != 0:
                        v_chains()
                    for j in range(NKV_L):
                        proj(wk_sb[:, j], k_sb, j)
                        if len(pending) > 1:
                            rope_flush()
                    if si == 0:
                        v_chains()
                    # block 3: heads 2-7 are deferred into phase B's light
                    # first s-block iterations (ScalarE-paced otherwise)
                    nq = 2 if si == NSB - 1 else NH_L
                    for h in range(nq):
                        proj(wq_slice(h), q_sb, h)
                        if len(pending) > 1:
                            rope_flush()
                while pending:
                    rope_flush()

            # ---------------- Phase B: attention + interleaved o_proj ------
            with (
                tc.tile_pool(name="wo", bufs=1) as wop,
                tc.tile_pool(name="attn", bufs=1) as ap_,
                tc.tile_pool(name="pblk", bufs=2) as pp,
                tc.tile_pool(name="accs", bufs=2) as accp,
                tc.tile_pool(name="tmps", bufs=2) as tmpp,
                tc.tile_pool(name="rcps", bufs=2) as rcpp,
                tc.tile_pool(name="flush", bufs=1) as flushp,
                tc.tile_pool(name="outp", bufs=2) as op_,
                tc.tile_pool(name="psc", bufs=2, space="PSUM") as pscp,
                tc.tile_pool(name="pat", bufs=1, space="PSUM") as patp,
                tc.tile_pool(name="pcs", bufs=1, space="PSUM") as pcsp,
                tc.tile_pool(name="po", bufs=2, space="PSUM") as pop,
            ):
                wo_sb = wop.tile([P, NH_L, HID], f16, tag="wo")
                wod = nc.sync.dma_start(out=wo_sb, in_=wo)
                add_dep_helper(
                    wod.ins, hs0_dma.ins, sync=True, reason="defer wo behind hs0"
                )
                # attnT ring over s-blocks: slot si%2 (o_proj of block si
                # finishes before block si+2 starts normalizing)
                attnT = ap_.tile([P, NH_L, 2, ST], f16, tag="attnT")

                # o_proj chain bookkeeping: one chain per (si, sj, ni);
                # ob rows accumulate 4 ni copies then DMA out per sj.
                ob_tiles = {}

                def o_chain(si, sj, ni):
                    po = pop.tile([P, ST], f32, tag="po")
                    for ft in range(NH_L):
                        nc.tensor.matmul(
                            po,
                            lhsT=attnT[:, ft, si % 2, sj * P : (sj + 1) * P],
                            rhs=wo_sb[:, ft, ni * ST : (ni + 1) * ST],
                            start=(ft == 0),
                            stop=(ft == NH_L - 1),
                        )
                    return po

                def o_finish(si, sj, ni, po):
                    st = si * (ST // P) + sj
                    if ni == 0:
                        ob_tiles[(si, sj)] = op_.tile(
                            [P, HID], f16, tag="ob", name="ob"
                        )
                    ob = ob_tiles[(si, sj)]
                    nc.vector.tensor_copy(ob[:, ni * ST : (ni + 1) * ST], po)
                    if si == NSB - 1 and sj >= 2:
                        # stream the last rows per-ni so the final DMA isn't
                        # serialized behind all four copybacks
                        nc.sync.dma_start(
                            out=out[st * P : (st + 1) * P, ni * ST : (ni + 1) * ST],
                            in_=ob[:, ni * ST : (ni + 1) * ST],
                        )
                        if ni == NSB - 1:
                            del ob_tiles[(si, sj)]
                    elif ni == NSB - 1:
                        nc.sync.dma_start(
                            out=out[st * P : (st + 1) * P, :], in_=ob
                        )
                        del ob_tiles[(si, sj)]

                # softmax denominator: DVE accumulates pairwise sums of the
                # exp tiles as they complete (latency after the last exp is
                # one pair-add + one acc-add), then a single all-ones matmul
                # reduces across partitions with the result replicated.
                def acc_pair(acc, pblk, k):
                    """add exp tile pair k into the running accumulator."""
                    if k == 0:
                        nc.vector.tensor_add(
                            acc, pblk[:, 0, :], pblk[:, 1, :]
                        )
                    else:
                        tmp = tmpp.tile([P, ST], f16, tag="tmp")
                        nc.vector.tensor_add(
                            tmp, pblk[:, 2 * k, :], pblk[:, 2 * k + 1, :]
                        )
                        nc.vector.tensor_add(acc, acc, tmp)

                def denom_reduce(acc):
                    """partition-sum of acc via ones-matmul; returns rcp."""
                    pcs = pcsp.tile([P, ST], f32, tag="pcs")
                    nc.tensor.matmul(pcs, lhsT=ones, rhs=acc, start=True, stop=True)
                    rcp = rcpp.tile([P, ST], f32, tag="rcp")
                    nc.vector.reciprocal_approx_fast(out=rcp, in_=pcs)
                    return rcp

                def normalize(h, si, pat, rcp):
                    nc.vector.tensor_mul(attnT[:, h, si % 2, :], pat, rcp)

                # iteration t: scores for blk[t], P@V+post for blk[t-1],
                # o_proj chains per OSCHED.
                blks = [(h, si) for si in range(NSB) for h in range(NH_L)]
                chains = {
                    si: [(si, sj, ni) for sj in range(ST // P) for ni in range(NSB)]
                    for si in range(NSB)
                }
                OSCHED = {t: [] for t in range(len(blks) + 1)}
                for si in range(NSB):
                    # chains 2c,2c+1 of o_proj(si) at iteration (si+1)*8+1+c,
                    # chains 14,15 at iteration (si+2)*8; overflow -> tail.
                    for c in range(7):
                        t = (si + 1) * 8 + 1 + c
                        OSCHED[min(t, len(blks))] += chains[si][2 * c : 2 * c + 2]
                    t = (si + 2) * 8
                    OSCHED[min(t, len(blks))] += chains[si][14:16]

                prev = None  # (h, si, pblk, acc)
                for t, (h, si) in enumerate(blks):
                    j = h // GROUP
                    s0 = si * ST
                    pblk = pp.tile([P, TT, ST], f16, tag="pblk")
                    acc = accp.tile([P, ST], f16, tag="acc")
                    ochains = OSCHED[t]
                    pat = None
                    rcp = None
                    po_a = po_b = None
                    for k in range(8):
                        # scores pair k -> exp
                        psc = pscp.tile([P, 2, ST], f32, tag="psc")
                        for u in range(2):
                            tt = 2 * k + u
                            nc.tensor.matmul(
                                psc[:, u, :],
                                lhsT=k_sb[:, j, tt * P : (tt + 1) * P],
                                rhs=q_sb[:, h, s0 : s0 + ST],
                                start=True,
                                stop=True,
                            )
                        nc.scalar.activation(
                            out=pblk[:, 2 * k : 2 * k + 2, :],
                            in_=psc,
                            func=EXP,
                            scale=SCALE,
                        )
                        # 2 P@V chain matmuls for the previous block
                        if prev is not None:
                            ph_, psi_, pblk_prev, acc_prev = prev
                            jp = ph_ // GROUP
                            if k == 0:
                                pat = patp.tile([P, ST], f32, tag="pat")
                                acc_pair(acc_prev, pblk_prev, 7)
                            for u in range(2):
                                tt = 2 * k + u
                                nc.tensor.matmul(
                                    pat,
                                    lhsT=v_sb[:, tt, jp * HD : (jp + 1) * HD],
                                    rhs=pblk_prev[:, tt, :],
                                    start=(tt == 0),
                                    stop=(tt == TT - 1),
                                )
                            if k == 2:
                                rcp = denom_reduce(acc_prev)
                            if k == 7:
                                normalize(ph_, psi_, pat, rcp)
                        # running denominator partial sums for this block
                        if k >= 1:
                            acc_pair(acc, pblk, k - 1)
                        # 2 o_proj chain matmuls (chain A at k<4, B at k>=4)
                        if ochains:
                            if k < 4:
                                if k == 0:
                                    po_a = pop.tile([P, ST], f32, tag="po")
                                ca = ochains[0]
                                for u in range(2):
                                    ft = 2 * k + u
                                    nc.tensor.matmul(
                                        po_a,
                                        lhsT=attnT[
                                            :, ft, ca[0] % 2,
                                            ca[1] * P : (ca[1] + 1) * P,
                                        ],
                                        rhs=wo_sb[
                                            :, ft, ca[2] * ST : (ca[2] + 1) * ST
                                        ],
                                        start=(ft == 0),
                                        stop=(ft == NH_L - 1),
                                    )
                                if k == 3:
                                    o_finish(*ochains[0], po_a)
                            elif len(ochains) > 1:
                                if k == 4:
                                    po_b = pop.tile([P, ST], f32, tag="po")
                                cb = ochains[1]
                                for u in range(2):
                                    ft = 2 * (k - 4) + u
                                    nc.tensor.matmul(
                                        po_b,
                                        lhsT=attnT[
                                            :, ft, cb[0] % 2,
                                            cb[1] * P : (cb[1] + 1) * P,
                                        ],
                                        rhs=wo_sb[
                                            :, ft, cb[2] * ST : (cb[2] + 1) * ST
                                        ],
                                        start=(ft == 0),
                                        stop=(ft == NH_L - 1),
                                    )
                                if k == 7:
                                    o_finish(*ochains[1], po_b)
                        # deferred block-3 q-projection chain (fills the
                        # otherwise ScalarE-paced first s-block iterations)
                        if 2 <= t <= 7:
                            hq = t
                            if k == 0:
                                pq = pop.tile([P, ST], f32, tag="po", name="pq")
                            for u in range(2):
                                kt = 2 * k + u
                                nc.tensor.matmul(
                                    pq,
                                    lhsT=wqhi_sb[:, hq - 2, kt, :],
                                    rhs=hs_blks[NSB - 1][:, kt, :],
                                    start=(kt == 0),
                                    stop=(kt == KT - 1),
                                )
                            if k == 7:
                                qc = flushp.tile([P, ST], f16, tag="fq")
                                nc.vector.tensor_copy(qc, pq)
                                rc = flushp.tile([P, ST], f16, tag="fr")
                                nc.vector.stream_shuffle(rc, qc, SHUF)
                                t1 = flushp.tile([P, ST], f16, tag="f1")
                                nc.vector.tensor_mul(
                                    t1, qc, cos_sb[:, (NSB - 1) * ST :]
                                )
                                t2 = flushp.tile([P, ST], f16, tag="f2")
                                nc.vector.tensor_mul(
                                    t2, rc, sin_sb[:, (NSB - 1) * ST :]
                                )
                                nc.vector.tensor_add(
                                    q_sb[:, hq, (NSB - 1) * ST :], t1, t2
                                )
                    prev = (h, si, pblk, acc)

                # ---- tail: P@V/post of last block + remaining o_proj ----
                ph_, psi_, pblk_prev, acc_prev = prev
                jp = ph_ // GROUP
                pat = patp.tile([P, ST], f32, tag="pat")
                acc_pair(acc_prev, pblk_prev, 7)
                rcp = denom_reduce(acc_prev)
                for tt in range(TT):
                    nc.tensor.matmul(
                        pat,
                        lhsT=v_sb[:, tt, jp * HD : (jp + 1) * HD],
                        rhs=pblk_prev[:, tt, :],
                        start=(tt == 0),
                        stop=(tt == TT - 1),
                    )
                normalize(ph_, psi_, pat, rcp)
                for ch in OSCHED[len(blks)]:
                    po = o_chain(*ch)
                    o_finish(*ch, po)

    nc.compile()
    return nc


def _get_nc():
    if "nc" not in _CACHE:
        _CACHE["nc"] = _build()
    return _CACHE["nc"]


def kernel(hidden_states, cos, sin, Wq, Wk, Wv, Wo):
    if "/opt/trn_rl_repo" not in sys.path:
        sys.path.insert(0, "/opt/trn_rl_repo")
    from concourse.bass_utils import run_bass_kernel_spmd

    hidden_states = np.asarray(hidden_states, dtype=np.float32)
    cos = np.asarray(cos, dtype=np.float32)
    sin = np.asarray(sin, dtype=np.float32)
    Wq = np.asarray(Wq, dtype=np.float32)
    Wk = np.asarray(Wk, dtype=np.float32)
    Wv = np.asarray(Wv, dtype=np.float32)
    Wo = np.asarray(Wo, dtype=np.float32)

    nc = _get_nc()
    dperm = np.concatenate(
        [np.r_[16 * q : 16 * q + 16, 64 + 16 * q : 64 + 16 * q + 16] for q in range(4)]
    )
    dsign = np.where(np.arange(HD) % 32 < 16, -1.0, 1.0).astype(np.float32)

    # pretiled host layouts: partition index first, contiguous per DMA slice
    def tile_khid(w):  # [HID, F] -> [P, KT, F]
        return np.ascontiguousarray(
            w.reshape(KT, P, w.shape[1]).transpose(1, 0, 2)
        ).astype(np.float16)

    in_maps = []
    hsT_b = [
        np.ascontiguousarray(
            hidden_states[b].T.reshape(KT, P, NSB, ST).transpose(1, 2, 0, 3)
        ).astype(np.float16)
        for b in range(B)
    ]
    cosT_b = [np.ascontiguousarray(cos[b].T[dperm]).astype(np.float16) for b in range(B)]
    sinT_b = [
        np.ascontiguousarray(sin[b].T[dperm] * dsign[:, None]).astype(np.float16)
        for b in range(B)
    ]
    for c in range(2 * B):
        b, half = c // 2, c % 2
        fq = slice(half * NH_L * HD, (half + 1) * NH_L * HD)
        fkv = slice(half * NKV_L * HD, (half + 1) * NKV_L * HD)
        wq_t = tile_khid(Wq[:, fq]).reshape(P, KT, NH_L, HD).transpose(0, 2, 1, 3)
        wq_t = wq_t[:, :, :, dperm]
        wk_t = tile_khid(Wk[:, fkv]).reshape(P, KT, NKV_L, HD).transpose(0, 2, 1, 3)
        wk_t = wk_t[:, :, :, dperm]
        wo_t = np.ascontiguousarray(
            Wo[fq, :].reshape(NH_L, P, HID).transpose(1, 0, 2)
        ).astype(np.float16)
        in_maps.append(
            {
                "hsT": hsT_b[b],
                "wq": np.ascontiguousarray(wq_t),
                "wk": np.ascontiguousarray(wk_t),
                "wv": tile_khid(Wv[:, fkv]),
                "wo": wo_t,
                "cosT": cosT_b[b],
                "sinT": sinT_b[b],
            }
        )

    res = run_bass_kernel_spmd(nc, in_maps, list(range(2 * B)))
    _CACHE["last_results"] = res

    out = np.empty((B, S, HID), dtype=np.float32)
    for b in range(B):
        out[b] = res.results[2 * b]["out"].astype(np.float32) + res.results[
            2 * b + 1
        ]["out"].astype(np.float32)
    return out
